# revision 39
# baseline (speedup 1.0000x reference)
"""Trainium2 Bass kernel for nn_KpcaStd (RBF-kernel PCA loss).

Computes, for x=input_data [8192,256], H [8192,512], D=inv_lambda_diag [512]:
    K = exp(-||x_i - x_j||^2 / 2)            [8192, 8192]
    E = H^T K                                 [512, 8192]
    s = -1/2 sum(D[:,None] * E^2) + 1/2 sum(E * H^T)
    out = s + 0.05 * s^2

Sharding: data-parallel over columns of K. Each of the 8 cores owns a
1024-column block K[:, c*1024:(c+1)*1024] (= rows c*1024.. of x), computes
the block, the partial E = H^T K_block [512, 1024], and per-partition
partial sums [128, 8]. The host sums partials across cores/partitions,
applies inv_lambda weights, and the final scalar map.

Device schedule per core (single pass over 64 i-chunks, engines balanced):
  G(ic): PSUM g = -2*x_i.x_j via one fp8 DoubleRow matmul per 512-col bank
    (contract 256). The sq_j term is added either on the PE (rank-4 fp8
    DoubleRow matmul of 2.0-weights x 4-way fp8 split of sq_j/2, zero-padded
    to 128 partitions; PESQ tiles) or on the DVE (g + sqb broadcast ->
    SBUF; remaining tiles) — PE and DVE are both near the roofline, so the
    64 adds are split between them.
    kt = Exp(-0.5*in + (-0.5*sq_i)) on ScalarE (fp32 bias, fp8 out), cached
    in SBUF as 32 pair tiles [128, 2048].
    sq is computed on host FROM THE fp8 x values, so the diagonal d2_ii
    cancels to ~1e-3 (K_ii = 1); off-diagonal d2 (~250+ in this regime)
    underflows exp to 0 in fp8 exactly like the f32 reference.
  E chunks interleaved into the same PE stream: per (block of kt pairs, hc)
  accumulate e[128,1024] over the block's pairs in PSUM via fp8 DoubleRow
  matmuls, then DVE-evict (add) into an SBUF f32 accumulator esb.
  PSUM: 2 g tiles (4 banks) + 2 e tiles (4 banks).
  Finals per hc: ACT Square+accum for sum E^2; GpSimd mul+reduce for
  sum E*H^T.
"""

import os
import sys

import numpy as np

sys.path.insert(0, "/opt/trn_rl_repo")

import ml_dtypes

import concourse.bacc as bacc
import concourse.mybir as mybir
import concourse.tile as tile
from concourse.bass_utils import run_bass_kernel_spmd

BF16 = mybir.dt.bfloat16
FP8 = mybir.dt.float8e4
F32 = mybir.dt.float32
NPBF16 = ml_dtypes.bfloat16
NPFP8 = ml_dtypes.float8_e4m3

N = 8192  # rows of K / x
D = 256  # feature dim
HD = 512  # columns of H
NCORES = 8
JS = N // NCORES  # 1024 columns of K per core
NI = N // 128  # 64 i-chunks
NP = NI // 2  # 32 i-chunk pairs
NH = HD // 128  # 4 h-blocks
DR = mybir.MatmulPerfMode.DoubleRow

# i-chunks whose sq_j add runs on the PE (rank-4 matmul); the rest go to
# the DVE. Balances PE vs DVE, which are both near the wall.
PESQ = frozenset((5, 16, 27, 38, 49, 60))

# E-accumulation blocks, in kt-pair units. 3 blocks x 4 hc = 12 PSUM->SBUF
# evictions on the DVE. Last block small so the post-G tail is short.
BLOCKS = [(0, 16), (16, 30), (30, 32)]

_cache = {}


def _build():
    """Build + schedule the single-core program (same on all 8 cores)."""
    nc = bacc.Bacc("TRN2", target_bir_lowering=False, debug=False)

    xtw_d = nc.dram_tensor("xtw", [NI, 128, D], FP8, kind="ExternalInput")
    xtr_d = nc.dram_tensor("xtr", [128, 2 * JS], FP8, kind="ExternalInput")
    r4_d = nc.dram_tensor("rfour", [2, 2 * JS], FP8, kind="ExternalInput")
    h_d = nc.dram_tensor("hmat", [NP, 128, 2 * HD], FP8, kind="ExternalInput")
    ht_d = nc.dram_tensor("htl", [HD, JS], BF16, kind="ExternalInput")
    sqb_d = nc.dram_tensor("sqb", [128, JS], F32, kind="ExternalInput")
    nb_d = nc.dram_tensor("nbias", [128, NI], F32, kind="ExternalInput")
    out_d = nc.dram_tensor("partials", [128, 2 * NH], F32, kind="ExternalOutput")

    with tile.TileContext(nc) as tc:
        with (
            tc.tile_pool(name="xw", bufs=NI) as xw_pool,
            tc.tile_pool(name="hp", bufs=NP) as h_pool,
            tc.tile_pool(name="kp", bufs=NP) as kt_pool,
            tc.tile_pool(name="cst", bufs=1) as cst_pool,
            tc.tile_pool(name="tmp", bufs=4) as tmp_pool,
            tc.tile_pool(name="gp", bufs=2, space="PSUM") as g_pool,
            tc.tile_pool(name="ep", bufs=2, space="PSUM") as e_pool,
        ):
            # xtr first on the gpsimd queue — it gates the first matmul.
            xtr = cst_pool.tile([128, 2 * JS], FP8)
            nc.gpsimd.dma_start(xtr[:], xtr_d.ap()[:])

            scr = cst_pool.tile([128, JS], F32)
            warm = cst_pool.tile([128, 1], F32)
            # rank-4 weights are 2.0 and rfour holds 4-way splits of sq/2,
            # keeping every split term under the 240 max of IEEE-style fp8
            # e4m3. Both operands zero-padded to 128 partitions so the PE
            # tile config matches the x matmuls.
            ones4 = cst_pool.tile([128, D], FP8)
            r4 = cst_pool.tile([128, 2 * JS], FP8)
            nc.gpsimd.memset(warm[:], 0.0)
            # pre-trigger the Exp table load during the DMA lead-in
            nc.scalar.activation(
                scr[:, 0:1], warm[:], mybir.ActivationFunctionType.Exp
            )
            nc.gpsimd.memset(ones4[:], 0.0)
            nc.gpsimd.memset(ones4[0:2, :], 2.0)
            nc.gpsimd.memset(r4[:], 0.0)
            nc.gpsimd.dma_start(r4[0:2, :], r4_d.ap()[:])

            nbias = cst_pool.tile([128, NI], F32)
            nc.gpsimd.dma_start(nbias[:], nb_d.ap()[:])
            sqb = cst_pool.tile([128, JS], F32)
            nc.gpsimd.dma_start(sqb[:], sqb_d.ap()[:])

            xw = []
            hts = []
            for ic in range(NI):
                w0 = xw_pool.tile([128, D], FP8, name=f"xw_{ic}", tag="xw")
                nc.sync.dma_start(w0[:], xtw_d.ap()[ic, :, :])
                xw.append(w0)
            for icp in range(NP):
                hh = h_pool.tile([128, 2 * HD], FP8, name=f"hch_{icp}", tag="hp")
                nc.gpsimd.dma_start(hh[:], h_d.ap()[icp, :, :])
                hts.append(hh)

            ht = cst_pool.tile([128, NH * JS], BF16)
            for hc in range(NH):
                nc.gpsimd.dma_start(
                    ht[:, hc * JS : (hc + 1) * JS],
                    ht_d.ap()[hc * 128 : (hc + 1) * 128, :],
                )

            esb = cst_pool.tile([128, NH * JS], F32)
            red = cst_pool.tile([128, 2 * NH], F32)

            xtrv = xtr[:].rearrange("p (ko j) -> p ko j", ko=2)
            r4v = r4[:].rearrange("p (ko j) -> p ko j", ko=2)
            ones4v = ones4[:].rearrange("p (ko m) -> p ko m", ko=2)
            kts = []
            for icp in range(NP):
                kt2 = kt_pool.tile([128, 2 * JS], FP8, name=f"kt_{icp}", tag="kt")
                kts.append(kt2)

            # --- E-chunk task stream (consumed between G chunks) -------
            # Each `yield gate` PRECEDES one emitted op; the op is emitted
            # by the next() that follows a passed gate. gate = min main-loop
            # ic at which the op may be emitted (kt pair p is written by
            # ics 2p, 2p+1; +3 ics of pipeline margin).
            def e_tasks():
                for b, (p0, p1) in enumerate(BLOCKS):
                    for hc in range(NH):
                        e = None
                        for p in range(p0, p1):
                            hv = hts[p][:].rearrange("p (ko f) -> p ko f", ko=2)
                            kv = kts[p][:].rearrange("p (ko j) -> p ko j", ko=2)
                            for jh in range(2):
                                sl = slice(jh * 512, jh * 512 + 512)
                                yield 2 * p + 4
                                if e is None:
                                    e = e_pool.tile(
                                        [128, JS], F32, name=f"e_{b}_{hc}", tag="ep"
                                    )
                                nc.tensor.matmul(
                                    e[:, sl],
                                    hv[:, :, hc * 128 : (hc + 1) * 128],
                                    kv[:, :, sl],
                                    start=(p == p0),
                                    stop=(p == p1 - 1),
                                    perf_mode=DR,
                                )
                        yield 2 * (p1 - 1) + 4
                        sle = slice(hc * JS, (hc + 1) * JS)
                        if b == 0:
                            nc.vector.tensor_copy(esb[:, sle], e[:])
                        else:
                            nc.vector.tensor_add(esb[:, sle], e[:], esb[:, sle])

            tasks = e_tasks()
            n_ops = 2 * NP * NH + len(BLOCKS) * NH
            state = {"pending": None, "done": False, "emitted": 0}

            def drain(ic, quota):
                if state["done"]:
                    return
                if state["pending"] is None:
                    try:
                        state["pending"] = next(tasks)
                    except StopIteration:
                        state["done"] = True
                        return
                while state["emitted"] < quota and (
                    ic is None or state["pending"] <= ic
                ):
                    try:
                        state["pending"] = next(tasks)
                        state["emitted"] += 1
                    except StopIteration:
                        state["done"] = True
                        return

            # --- main G loop -------------------------------------------
            for ic in range(NI):
                g = g_pool.tile([128, JS], F32, name=f"g_{ic}", tag="gp")
                wv = xw[ic][:].rearrange("p (ko m) -> p ko m", ko=2)
                pe_sq = ic in PESQ
                for jh in range(2):
                    sl = slice(jh * 512, jh * 512 + 512)
                    if pe_sq:
                        nc.tensor.matmul(
                            g[:, sl], ones4v[:, :, :], r4v[:, :, sl],
                            start=True, stop=False, perf_mode=DR,
                        )
                    nc.tensor.matmul(
                        g[:, sl], wv[:, :, :], xtrv[:, :, sl],
                        start=not pe_sq, stop=True, perf_mode=DR,
                    )
                ko = ic % 2
                dst = kts[ic // 2][:, ko * JS : (ko + 1) * JS]
                if pe_sq:
                    nc.scalar.activation(
                        dst, g[:],
                        mybir.ActivationFunctionType.Exp,
                        bias=nbias[:, ic : ic + 1],
                        scale=-0.5,
                    )
                else:
                    ta = tmp_pool.tile([128, JS], F32, name=f"ta_{ic}", tag="tmp")
                    nc.vector.tensor_add(ta[:], g[:], sqb[:])
                    nc.scalar.activation(
                        dst, ta[:],
                        mybir.ActivationFunctionType.Exp,
                        bias=nbias[:, ic : ic + 1],
                        scale=-0.5,
                    )
                if ic >= 4:
                    drain(ic, (n_ops * (ic - 3)) // 60)
            drain(None, 1 << 30)

            # --- final reductions --------------------------------------
            # Tail-latency critical: spread the per-hc loss2 mul+reduce
            # chains across DVE (hc 0/1) and GpSimd+ACT (hc 2/3) so they
            # run concurrently with the ACT squares.
            t2 = cst_pool.tile([128, JS], F32)
            t34 = [
                cst_pool.tile([128, JS], F32, name=f"t34_{i}") for i in range(2)
            ]
            for hc in range(NH):
                sle = slice(hc * JS, (hc + 1) * JS)
                nc.scalar.activation(
                    scr[:], esb[:, sle],
                    mybir.ActivationFunctionType.Square,
                    accum_out=red[:, hc : hc + 1],
                )
                if hc < 2:
                    nc.vector.tensor_mul(t2[:], esb[:, sle], ht[:, sle])
                    nc.vector.reduce_sum(
                        red[:, NH + hc : NH + hc + 1], t2[:],
                        axis=mybir.AxisListType.X,
                    )
                else:
                    t3 = t34[hc - 2]
                    nc.gpsimd.tensor_mul(t3[:], esb[:, sle], ht[:, sle])
                    nc.scalar.activation(
                        scr[:], t3[:],
                        mybir.ActivationFunctionType.Copy,
                        accum_out=red[:, NH + hc : NH + hc + 1],
                    )

            nc.sync.dma_start(out_d.ap()[:], red[:])

    nc.compile()
    return nc


def _fp8_split4(v):
    """4-term fp8 split of v (f32): terms sum to v within ~2^-11."""
    terms = []
    r = v.astype(np.float32)
    for _ in range(4):
        t = r.astype(NPFP8)
        terms.append(t)
        r = r - t.astype(np.float32)
    return terms


def _prep_inputs(input_data, H, inv_lambda_diag):
    x32 = np.asarray(input_data, dtype=np.float32)
    xq = x32.astype(NPFP8)
    xqf = xq.astype(np.float32)
    # row norms of the *fp8* x in fp64->fp32: the PE's G_ii equals this up
    # to fp32 accumulation order, so the diagonal of d2 cancels to ~0.
    sq = (xqf.astype(np.float64) ** 2).sum(axis=1).astype(np.float32)

    # DoubleRow weights: xtw[ic, p, ko*128+m] = fp8(x)[ic*128+m, ko*128+p]
    xtw = np.ascontiguousarray(
        xqf.reshape(NI, 128, 2, 128).transpose(0, 3, 2, 1).reshape(NI, 128, D)
    ).astype(NPFP8)
    h8f = np.asarray(H, dtype=np.float32).astype(NPFP8).astype(np.float32)
    # H pairs: hmat[icp, p, ko*512+f] = fp8(H)[(2*icp+ko)*128+p, f]
    hp2 = np.ascontiguousarray(
        h8f.reshape(NP, 2, 128, HD).transpose(0, 2, 1, 3).reshape(NP, 128, 2 * HD)
    ).astype(NPFP8)
    nbias = np.ascontiguousarray((-0.5 * sq).reshape(NI, 128).T).astype(
        np.float32
    )

    in_maps = []
    for c in range(NCORES):
        sl = slice(c * JS, (c + 1) * JS)
        # xtr[p, ko*1024+j] = -2*fp8(x)[c*1024+j, ko*128+p]
        xtr = np.ascontiguousarray(
            (-2.0 * xqf[sl]).T.reshape(2, 128, JS).transpose(1, 0, 2).reshape(128, 2 * JS)
        ).astype(NPFP8)
        # rfour[p, ko*1024+j] = split_{2p+ko}(sq[c*1024+j] / 2); the 2x
        # comes back via the 2.0 weights (exact in fp8 products).
        sp = _fp8_split4(0.5 * sq[sl])
        r4 = np.ascontiguousarray(
            np.stack(sp).reshape(2, 2, JS).reshape(2, 2 * JS)
        ).astype(NPFP8)
        sqb = np.ascontiguousarray(
            np.broadcast_to(sq[sl], (128, JS))
        ).astype(np.float32)
        htl = np.ascontiguousarray(
            np.asarray(H, dtype=np.float32)[sl].T
        ).astype(NPBF16)
        in_maps.append(
            {
                "xtw": xtw,
                "xtr": xtr,
                "rfour": r4,
                "hmat": hp2,
                "htl": htl,
                "sqb": sqb,
                "nbias": nbias,
            }
        )
    return in_maps


def kernel(input_data, H, inv_lambda_diag, _want_profile=False):
    if "nc" not in _cache:
        _cache["nc"] = _build()
    nc = _cache["nc"]
    in_maps = _prep_inputs(input_data, H, inv_lambda_diag)

    trace = bool(_want_profile or os.environ.get("KPCA_TRACE"))
    res = run_bass_kernel_spmd(
        nc, in_maps, list(range(NCORES)), trace=trace,
        tmpdir=os.environ.get("KPCA_TRACE_DIR") or None,
    )
    _cache["last_result"] = res

    dv = np.asarray(inv_lambda_diag, dtype=np.float64).reshape(NH, 128).T
    s1 = 0.0
    s2 = 0.0
    for c in range(NCORES):
        parts = res.results[c]["partials"].astype(np.float64)
        s1 += (dv * parts[:, :NH]).sum()
        s2 += parts[:, NH:].sum()
    s = -0.5 * s1 + 0.5 * s2
    out = s + 0.05 * s * s
    return np.array(out, dtype=np.float32)
